# revision 23
# baseline (speedup 1.0000x reference)
"""Trainium2 Bass kernel: bidirectional-LSTM final-cell-state encoder.

Problem: 4 independent BasicLSTMCell chains (premise fw/bw, hypothesis fw/bw),
B=1024, T=128, D=300, H=100.  Output [B, 4H] = concat of final cell states.

Strategy (v9)
-------------
* Data-parallel: batch sharded 8 ways -> 128 rows/core; each core runs the 4
  chains as 2 pairs (premise fw+bw, hypothesis fw+bw) of per-run recurrences.
  The kernel is latency-bound on the per-step recurrence cycle (~3.9us).
* z_t [128b, 400g] per run accumulated in PSUM (pair tile [128, 1024] = 2
  banks, double buffered = 8 banks) from 3 pre-transposed bf16 x chunks + 1
  recurrent h^T chunk.  x chunks stream from DRAM with one-block prefetch;
  block 0 and the weights are split across queues to shorten the prologue.
* TF gate order (i, j, f, o) kept; j columns pre-scaled by 2 so sigmoid
  covers tanh(zj) == 2*sigmoid(2*zj) - 1.
* The recurrent matmul and the sigmoid are split by gate group: h@Wh for
  the {i,j,f} columns (N=300) completes first and unblocks sigmoid_a
  {i,j,f}; the {o} columns + sigmoid_b run in the shadow (o is only needed
  at the very end of the chain, for h).
* sigmoid_a writes S in GATE-MAJOR layout [i|j|f|o][run0|run1][100] via a
  strided dst AP (free permute inside the ACT instruction) so downstream
  DVE ops read flat 2D slices.
* Cell state kept as c' = c/2 in bf16 (fp32 only for the final step's
  output) so the tanh identity constants fold away and the DVE ops hit the
  2x 16-bit perf mode:
    P' = (sig(2zj) - 0.5) * sig(zi)      [DVE scalar_tensor_tensor]
    F  = sig(zf) * c'                    [DVE tensor_tensor, 2x]
    c2' = F + P'                         [DVE tensor_tensor, 2x]
    tc = tanh(2 * c2')                   [ACT, scale=2]
    h  = tc * sig(zo)                    [DVE tensor_tensor, 2x]
  Host multiplies the gathered output by 2.
* h^T: bf16 PE transpose into dead PSUM columns of the z tile, evacuated by
  DVE; transpose+evac of step t-1 are issued between step t's x-matmuls and
  h-matmuls so the PE FIFO never parks a dependent transpose in front of
  independent x-projection work.
"""

import numpy as np

B, T, D, H = 1024, 128, 300, 100
NCORES = 8
BL = B // NCORES          # 128 batch rows per core
G4 = 4 * H                # 400 gate columns
KCH = 128                 # 100 d-rows + 1 ones-row + zero-pad (FWL needs K=128)
TB = 8                    # timesteps per DMA block
FORGET_BIAS = 1.0

_CACHE = {}


def _build_program(n_steps=T):
    from contextlib import ExitStack

    import concourse.mybir as mybir
    import concourse.tile as tile
    from concourse import bacc

    f32 = mybir.dt.float32
    bf16 = mybir.dt.bfloat16
    fp16 = mybir.dt.float16
    Sig = mybir.ActivationFunctionType.Sigmoid
    Tanh = mybir.ActivationFunctionType.Tanh
    mult = mybir.AluOpType.mult
    add = mybir.AluOpType.add

    nc = bacc.Bacc(
        "TRN2",
        target_bir_lowering=False,
        debug=False,
        enable_asserts=False,
        num_devices=NCORES,
    )

    xt_p = nc.dram_tensor("xt_p", [T // TB, KCH, TB * 3 * BL], bf16, kind="ExternalInput").ap()
    xt_h = nc.dram_tensor("xt_h", [T // TB, KCH, TB * 3 * BL], bf16, kind="ExternalInput").ap()
    w_all = nc.dram_tensor("w_all", [KCH, 16 * G4], bf16, kind="ExternalInput").ap()
    wh_bf = nc.dram_tensor("wh_bf", [128, 4 * G4], bf16, kind="ExternalInput").ap()
    ident = nc.dram_tensor("ident", [128, 128], bf16, kind="ExternalInput").ap()
    out = nc.dram_tensor("out", [BL, G4], f32, kind="ExternalOutput").ap()

    with tile.TileContext(nc) as tc, ExitStack() as ctx:
        w_sb = nc.alloc_sbuf_tensor("w_sb", [KCH, 16 * G4], bf16).ap()
        wh_sb = nc.alloc_sbuf_tensor("wh_sb", [128, 4 * G4], bf16).ap()
        id_sb = nc.alloc_sbuf_tensor("id_sb", [128, 128], bf16).ap()

        # per-pair tensors; S and HH double-buffered (cross-engine WAR)
        # S layout is GATE-MAJOR: [i(200) | f(200) | o(200) | j(200)],
        # each gate block = [run0(100) | run1(100)].
        S, PP, FF, CC, TC, HH = [], [], [], [], [], []
        for p in range(2):
            S.append([nc.alloc_sbuf_tensor(f"s{p}_{b}", [BL, 800], bf16).ap() for b in range(2)])
            PP.append(nc.alloc_sbuf_tensor(f"pp{p}", [BL, 200], bf16).ap())
            FF.append(nc.alloc_sbuf_tensor(f"ff{p}", [BL, 200], bf16).ap())
            CC.append(nc.alloc_sbuf_tensor(f"cc{p}", [BL, 200], bf16).ap())
            TC.append(nc.alloc_sbuf_tensor(f"tc{p}", [BL, 200], bf16).ap())
            HH.append([nc.alloc_sbuf_tensor(f"hh{p}_{b}", [BL, 200], bf16).ap() for b in range(2)])
        HT = [nc.alloc_sbuf_tensor(f"ht{p}", [128, 256], bf16).ap() for p in range(2)]
        CCF = [nc.alloc_sbuf_tensor(f"ccf{p}", [BL, 200], f32).ap() for p in range(2)]

        # weights split per run across both HWDGE queues so the first
        # step's matmuls unblock as early as possible
        for r in range(4):
            eng = nc.sync if r % 2 == 0 else nc.scalar
            eng.dma_start(
                w_sb[:, r * 4 * G4 : (r + 1) * 4 * G4],
                w_all[:, r * 4 * G4 : (r + 1) * 4 * G4],
            )
        for p in range(2):
            nc.vector.memset(CC[p], 0.0)
        for p in range(2):
            nc.vector.memset(HT[p], 0.0)

        xt_pools = [
            ctx.enter_context(tc.tile_pool(name=f"xt{s}", bufs=2)) for s in range(4)
        ]
        zpools = [
            ctx.enter_context(tc.tile_pool(name=f"zp{p}", bufs=2, space="PSUM"))
            for p in range(2)
        ]

        # stream s: (dram tensor, reversed?) for runs (p_fw, p_bw, h_fw, h_bw)
        streams = [(xt_p, False), (xt_p, True), (xt_h, False), (xt_h, True)]
        cur = [None] * 4
        nxt = [None] * 4

        def dma_block(bi, into):
            for s, (dram, rev) in enumerate(streams):
                tl = xt_pools[s].tile(
                    [KCH, TB * 3 * 128], bf16, tag=f"x{s}", name=f"x{s}_b{bi}"
                )
                nblk = (T // TB - 1 - bi) if rev else bi
                if bi == 0:
                    # split block 0 so the first-consumed timesteps arrive
                    # first (fw runs read tq=0.., bw runs read tq=7..),
                    # spread across both HWDGE queues
                    eng = nc.sync if s % 2 == 0 else nc.scalar
                    cut = 2 * 3 * 128 if not rev else 6 * 3 * 128
                    first = slice(0, cut) if not rev else slice(cut, TB * 3 * 128)
                    rest = slice(cut, TB * 3 * 128) if not rev else slice(0, cut)
                    eng.dma_start(tl[:, first], dram[nblk][:, first])
                    eng.dma_start(tl[:, rest], dram[nblk][:, rest])
                else:
                    nc.sync.dma_start(tl[:, :], dram[nblk])
                into[s] = tl

        dma_block(0, cur)
        nc.scalar.dma_start(wh_sb, wh_bf)
        nc.sync.dma_start(id_sb, ident)

        prev_z = None  # previous step's z tiles (for deferred transpose+evac)

        for t in range(n_steps):
            if t % TB == 0:
                if t > 0:
                    cur, nxt = nxt, [None] * 4
                if t + TB < n_steps:
                    dma_block(t // TB + 1, nxt)
            sb = t % 2   # S/HH buffer parity

            z = [
                zpools[p].tile([BL, 1024], f32, tag=f"z{p}", name=f"z{p}_{t}")
                for p in range(2)
            ]
            # Each pair's entire step is emitted contiguously: the pairs run
            # skewed by ~half a period, and every engine's FIFO is strict
            # in-order, so grouping per pair keeps FIFO order aligned with
            # readiness order (no cross-pair head-of-line blocking).
            for p in range(2):
                # x-projection matmuls first: independent of the recurrence,
                # they fill the PE while the elementwise chain runs
                for rh in range(2):
                    r = p * 2 + rh
                    rev = streams[r][1]
                    tq = (TB - 1 - t % TB) if rev else (t % TB)
                    tl = cur[r]
                    for k in range(3):
                        nc.tensor.matmul(
                            z[p][:, rh * 512 : rh * 512 + G4],
                            tl[:, (tq * 3 + k) * 128 : (tq * 3 + k + 1) * 128],
                            w_sb[:, (r * 4 + k) * G4 : (r * 4 + k + 1) * G4],
                            start=(k == 0),
                            stop=(t == 0 and k == 2),
                            skip_group_check=True,
                        )
                if prev_z is not None:
                    # transpose h(t-1) into dead PSUM cols + evac to SBUF,
                    # between x(t) and h(t) in the PE FIFO
                    for rh in range(2):
                        nc.tensor.transpose(
                            prev_z[p][0:H, rh * 512 + 400 : rh * 512 + 464].bitcast(bf16),
                            HH[p][1 - sb][:, rh * 100 : rh * 100 + 100],
                            id_sb,
                        )
                    hsrc = (
                        prev_z[p][0:H, :]
                        .bitcast(bf16)
                        .rearrange("q (r c) -> q r c", r=2)[:, :, 800:928]
                    )
                    nc.vector.tensor_copy(
                        HT[p][0:H, :].rearrange("q (r c) -> q r c", r=2), hsrc
                    )
                # recurrent matmuls split by gate group: the {i,j,f} columns
                # finish first so the critical-path sigmoid starts while the
                # {o} columns (only needed much later, at HH) still stream
                # (skipped at t=0: h(-1) == 0, z(0) is the x projection alone)
                for rh in range(2) if t > 0 else []:
                    r = p * 2 + rh
                    nc.tensor.matmul(
                        z[p][:, rh * 512 : rh * 512 + 300],
                        HT[p][0:H, rh * 128 : rh * 128 + 128],
                        wh_sb[0:H, r * G4 : r * G4 + 300],
                        start=False,
                        stop=True,
                        skip_group_check=True,
                    )
                for rh in range(2) if t > 0 else []:
                    r = p * 2 + rh
                    nc.tensor.matmul(
                        z[p][:, rh * 512 + 300 : rh * 512 + G4],
                        HT[p][0:H, rh * 128 : rh * 128 + 128],
                        wh_sb[0:H, r * G4 + 300 : (r + 1) * G4],
                        start=False,
                        stop=True,
                        skip_group_check=True,
                    )

                # sigmoid covers all 4 gates (j cols pre-doubled in W); split
                # {i,j,f} / {o} so the c-path starts early; dst AP permutes
                # run-major z -> gate-major S for flat DVE reads
                za = (
                    z[p][:, :]
                    .rearrange("b (r x) -> b r x", r=2)[:, :, 0:300]
                    .rearrange("b r (g c) -> b r g c", g=3)
                )
                sa = (
                    S[p][sb][:, 0:600]
                    .rearrange("b (g r c) -> b r g c", g=3, r=2)
                )
                nc.scalar.activation(sa, za, Sig)
                Sg = S[p][sb]
                # P' = (sig(2zj) - 0.5) * sig(zi)
                nc.vector.scalar_tensor_tensor(
                    PP[p], Sg[:, 200:400], -0.5, Sg[:, 0:200], add, mult
                )
                # F = sig(zf) * c'
                nc.vector.tensor_tensor(FF[p], Sg[:, 400:600], CC[p], mult)
                zb = z[p][:, :].rearrange("b (r x) -> b r x", r=2)[:, :, 300:400]
                sob = Sg[:, 600:800].rearrange("b (r c) -> b r c", r=2)
                nc.scalar.activation(sob, zb, Sig)
                # c2' = F + P'
                nc.vector.tensor_tensor(CC[p], FF[p], PP[p], add)

                if t == n_steps - 1:
                    nc.vector.tensor_tensor(CCF[p], FF[p], PP[p], add)
                    nc.sync.dma_start(out[:, p * 200 : (p + 1) * 200], CCF[p])
                    continue

                # tc = tanh(2*c') = tanh(c)
                nc.scalar.activation(TC[p], CC[p], Tanh, scale=2.0)
                # h = tc * sig(zo)
                nc.vector.tensor_tensor(
                    HH[p][sb], TC[p], Sg[:, 600:800], mult
                )
            if t == n_steps - 1:
                break
            prev_z = z

    nc.compile()
    return nc


def _prep_xt(x_slice):
    """[BL, T, D] fp32 -> [T//TB, 101, TB*3*BL] bf16 block-major tiles.

    tile[n, p, (tq, j, b)] = x[b, n*TB+tq, j*100+p] for p<100; p=100 is the
    baked-in ones row (bias trick).  Each DMA block is a plain 2D copy with
    TB*3*BL*2 contiguous bytes per partition.
    """
    import ml_dtypes

    a = x_slice.transpose(1, 2, 0).reshape(T // TB, TB, 3, 100, BL)
    a = a.transpose(0, 3, 1, 2, 4)  # [n, p, tq, j, b]
    outp = np.zeros((T // TB, KCH, TB, 3, BL), ml_dtypes.bfloat16)
    outp[:, :100] = a.astype(ml_dtypes.bfloat16)
    outp[:, 100] = 1.0
    return outp.reshape(T // TB, KCH, TB * 3 * BL)


def _prep_weights(Ws, bs):
    """Pack 4 runs' [D+H, 4H] weights into [128, 16*400] chunk blocks.

    Gate columns permuted (i,j,f,o) -> (i,f,o,j); the j block (cols 300:400
    after the permute) is scaled by 2 so tanh(zj) = 2*sigmoid(2 zj) - 1 comes
    out of one sigmoid pass.  Chunk-2's row 100 carries the permuted bias
    (+1.0 forget bias on the f block, x2 on the j block).  Also emits the
    recurrent rows (300:400) as bf16 [128, 4*400].
    """
    import ml_dtypes

    perm = np.arange(400)  # TF gate order (i,j,f,o) kept as-is
    w_all = np.zeros((KCH, 16 * G4), ml_dtypes.bfloat16)
    wh_bf = np.zeros((128, 4 * G4), ml_dtypes.bfloat16)
    for u in range(4):
        Wp = np.asarray(Ws[u], np.float32)[:, perm].copy()
        Wp[:, 100:200] *= 2.0
        bp = np.asarray(bs[u], np.float32)[perm].copy()
        bp[100:200] *= 2.0
        for k in range(3):
            blk = w_all[:, (u * 4 + k) * G4 : (u * 4 + k + 1) * G4]
            blk[0:100] = Wp[k * 100 : (k + 1) * 100].astype(ml_dtypes.bfloat16)
        bias_row = bp.copy()
        bias_row[200:300] += FORGET_BIAS
        w_all[100, (u * 4 + 2) * G4 : (u * 4 + 3) * G4] = bias_row.astype(
            ml_dtypes.bfloat16
        )
        wh_bf[0:H, u * G4 : (u + 1) * G4] = Wp[300:400].astype(ml_dtypes.bfloat16)
    return w_all, wh_bf


def _prep_in_maps(inputs):
    import ml_dtypes

    w_all, wh_bf = _prep_weights(
        [inputs["Wp_fw"], inputs["Wp_bw"], inputs["Wh_fw"], inputs["Wh_bw"]],
        [inputs["bp_fw"], inputs["bp_bw"], inputs["bh_fw"], inputs["bh_bw"]],
    )
    ident = np.eye(128, dtype=ml_dtypes.bfloat16)

    in_maps = []
    for c in range(NCORES):
        sl = slice(c * BL, (c + 1) * BL)
        in_maps.append(
            {
                "xt_p": _prep_xt(np.asarray(inputs["premises"][sl], np.float32)),
                "xt_h": _prep_xt(np.asarray(inputs["hypotheses"][sl], np.float32)),
                "w_all": w_all,
                "wh_bf": wh_bf,
                "ident": ident,
            }
        )
    return in_maps


def _post(res):
    # columns are (c_pf, c_pb, c_hf, c_hb) in run order already; state is c/2
    out = np.concatenate([r["out"] for r in res.results], axis=0)
    return 2.0 * out


def kernel(premises, hypotheses, Wp_fw, bp_fw, Wp_bw, bp_bw, Wh_fw, bh_fw, Wh_bw, bh_bw):
    from concourse.bass_utils import run_bass_kernel_spmd

    if "nc" not in _CACHE:
        _CACHE["nc"] = _build_program()
    nc = _CACHE["nc"]

    in_maps = _prep_in_maps(
        {
            "premises": premises,
            "hypotheses": hypotheses,
            "Wp_fw": Wp_fw, "bp_fw": bp_fw,
            "Wp_bw": Wp_bw, "bp_bw": bp_bw,
            "Wh_fw": Wh_fw, "bh_fw": bh_fw,
            "Wh_bw": Wh_bw, "bh_bw": bh_bw,
        }
    )
    res = run_bass_kernel_spmd(nc, in_maps, core_ids=list(range(NCORES)))
    return _post(res)


# revision 26
# speedup vs baseline: 1.0067x; 1.0067x over previous
"""Trainium2 Bass kernel: bidirectional-LSTM final-cell-state encoder.

Problem: 4 independent BasicLSTMCell chains (premise fw/bw, hypothesis fw/bw),
B=1024, T=128, D=300, H=100.  Output [B, 4H] = concat of final cell states.

Strategy (v9)
-------------
* Data-parallel: batch sharded 8 ways -> 128 rows/core; each core runs the 4
  chains as 2 pairs (premise fw+bw, hypothesis fw+bw) of per-run recurrences.
  The kernel is latency-bound on the per-step recurrence cycle (~3.9us).
* z_t [128b, 400g] per run accumulated in PSUM (pair tile [128, 1024] = 2
  banks, double buffered = 8 banks) from 3 pre-transposed bf16 x chunks + 1
  recurrent h^T chunk.  x chunks stream from DRAM with one-block prefetch;
  block 0 and the weights are split across queues to shorten the prologue.
* TF gate order (i, j, f, o) kept; j columns pre-scaled by 2 so sigmoid
  covers tanh(zj) == 2*sigmoid(2*zj) - 1.
* The recurrent matmul and the sigmoid are split by gate group: h@Wh for
  the {i,j,f} columns (N=300) completes first and unblocks sigmoid_a
  {i,j,f}; the {o} columns + sigmoid_b run in the shadow (o is only needed
  at the very end of the chain, for h).
* sigmoid_a writes S in GATE-MAJOR layout [i|j|f|o][run0|run1][100] via a
  strided dst AP (free permute inside the ACT instruction) so downstream
  DVE ops read flat 2D slices.
* Cell state kept as c' = c/2 in bf16 (fp32 only for the final step's
  output) so the tanh identity constants fold away and the DVE ops hit the
  2x 16-bit perf mode:
    P' = (sig(2zj) - 0.5) * sig(zi)      [DVE scalar_tensor_tensor]
    F  = sig(zf) * c'                    [DVE tensor_tensor, 2x]
    c2' = F + P'                         [DVE tensor_tensor, 2x]
    tc = tanh(2 * c2')                   [ACT, scale=2]
    h  = tc * sig(zo)                    [DVE tensor_tensor, 2x]
  Host multiplies the gathered output by 2.
* h^T: bf16 PE transpose into dead PSUM columns of the z tile, evacuated by
  DVE; transpose+evac of step t-1 are issued between step t's x-matmuls and
  h-matmuls so the PE FIFO never parks a dependent transpose in front of
  independent x-projection work.
"""

import numpy as np

B, T, D, H = 1024, 128, 300, 100
NCORES = 8
BL = B // NCORES          # 128 batch rows per core
G4 = 4 * H                # 400 gate columns
KCH = 128                 # 100 d-rows + 1 ones-row + zero-pad (FWL needs K=128)
TB = 8                    # timesteps per DMA block
FORGET_BIAS = 1.0

_CACHE = {}


def _build_program(n_steps=T):
    from contextlib import ExitStack

    import concourse.mybir as mybir
    import concourse.tile as tile
    from concourse import bacc

    f32 = mybir.dt.float32
    bf16 = mybir.dt.bfloat16
    fp16 = mybir.dt.float16
    Sig = mybir.ActivationFunctionType.Sigmoid
    Tanh = mybir.ActivationFunctionType.Tanh
    mult = mybir.AluOpType.mult
    add = mybir.AluOpType.add

    nc = bacc.Bacc(
        "TRN2",
        target_bir_lowering=False,
        debug=False,
        enable_asserts=False,
        num_devices=NCORES,
    )

    xt_p = nc.dram_tensor("xt_p", [T // TB, KCH, TB * 3 * BL], bf16, kind="ExternalInput").ap()
    xt_h = nc.dram_tensor("xt_h", [T // TB, KCH, TB * 3 * BL], bf16, kind="ExternalInput").ap()
    w_all = nc.dram_tensor("w_all", [KCH, 16 * G4], bf16, kind="ExternalInput").ap()
    wh_bf = nc.dram_tensor("wh_bf", [128, 4 * G4], bf16, kind="ExternalInput").ap()
    ident = nc.dram_tensor("ident", [128, 128], bf16, kind="ExternalInput").ap()
    out = nc.dram_tensor("out", [BL, G4], f32, kind="ExternalOutput").ap()

    with tile.TileContext(nc) as tc, ExitStack() as ctx:
        w_sb = nc.alloc_sbuf_tensor("w_sb", [KCH, 16 * G4], bf16).ap()
        wh_sb = nc.alloc_sbuf_tensor("wh_sb", [128, 4 * G4], bf16).ap()
        id_sb = nc.alloc_sbuf_tensor("id_sb", [128, 128], bf16).ap()

        # per-pair tensors; S and HH double-buffered (cross-engine WAR)
        # S layout is GATE-MAJOR: [i(200) | f(200) | o(200) | j(200)],
        # each gate block = [run0(100) | run1(100)].
        S, PP, FF, CC, TC, HH = [], [], [], [], [], []
        for p in range(2):
            S.append([nc.alloc_sbuf_tensor(f"s{p}_{b}", [BL, 800], bf16).ap() for b in range(2)])
            PP.append(nc.alloc_sbuf_tensor(f"pp{p}", [BL, 200], bf16).ap())
            FF.append(nc.alloc_sbuf_tensor(f"ff{p}", [BL, 200], bf16).ap())
            CC.append(nc.alloc_sbuf_tensor(f"cc{p}", [BL, 200], bf16).ap())
            TC.append(nc.alloc_sbuf_tensor(f"tc{p}", [BL, 200], bf16).ap())
            HH.append([nc.alloc_sbuf_tensor(f"hh{p}_{b}", [BL, 200], bf16).ap() for b in range(2)])
        HT = [nc.alloc_sbuf_tensor(f"ht{p}", [128, 256], bf16).ap() for p in range(2)]
        CCF = [nc.alloc_sbuf_tensor(f"ccf{p}", [BL, 200], f32).ap() for p in range(2)]

        for p in range(2):
            nc.vector.memset(CC[p], 0.0)
        for p in range(2):
            nc.vector.memset(HT[p], 0.0)

        xt_pools = [
            ctx.enter_context(tc.tile_pool(name=f"xt{s}", bufs=2)) for s in range(4)
        ]
        zpools = [
            ctx.enter_context(tc.tile_pool(name=f"zp{p}", bufs=2, space="PSUM"))
            for p in range(2)
        ]

        # stream s: (dram tensor, reversed?) for runs (p_fw, p_bw, h_fw, h_bw)
        streams = [(xt_p, False), (xt_p, True), (xt_h, False), (xt_h, True)]
        cur = [None] * 4
        nxt = [None] * 4

        def dma_block(bi, into):
            for s, (dram, rev) in enumerate(streams):
                tl = xt_pools[s].tile(
                    [KCH, TB * 3 * 128], bf16, tag=f"x{s}", name=f"x{s}_b{bi}"
                )
                nblk = (T // TB - 1 - bi) if rev else bi
                nc.sync.dma_start(tl[:, :], dram[nblk])
                into[s] = tl

        # ── prologue DMAs in priority order across both HWDGE queues ──
        # pair A's step-0 data (w run0/run1 + first-consumed x slices of
        # streams 0/1) first, then pair B's, then the bulk + wh/id.  fw runs
        # read block-0 tq=0.., bw runs read tq=7.., so block 0 is split.
        b0 = []
        for s, (dram, rev) in enumerate(streams):
            tl = xt_pools[s].tile(
                [KCH, TB * 3 * 128], bf16, tag=f"x{s}", name=f"x{s}_b0"
            )
            nblk = (T // TB - 1) if rev else 0
            cut = 2 * 3 * 128 if not rev else 6 * 3 * 128
            first = slice(0, cut) if not rev else slice(cut, TB * 3 * 128)
            rest = slice(cut, TB * 3 * 128) if not rev else slice(0, cut)
            b0.append((tl, dram[nblk], first, rest))
            cur[s] = tl

        def wdma(eng, r):
            eng.dma_start(
                w_sb[:, r * 4 * G4 : (r + 1) * 4 * G4],
                w_all[:, r * 4 * G4 : (r + 1) * 4 * G4],
            )

        wdma(nc.sync, 0)
        wdma(nc.scalar, 1)
        nc.sync.dma_start(b0[0][0][:, b0[0][2]], b0[0][1][:, b0[0][2]])
        nc.scalar.dma_start(b0[1][0][:, b0[1][2]], b0[1][1][:, b0[1][2]])
        wdma(nc.sync, 2)
        wdma(nc.scalar, 3)
        nc.sync.dma_start(b0[2][0][:, b0[2][2]], b0[2][1][:, b0[2][2]])
        nc.scalar.dma_start(b0[3][0][:, b0[3][2]], b0[3][1][:, b0[3][2]])
        for s in range(4):
            eng = nc.sync if s % 2 == 0 else nc.scalar
            eng.dma_start(b0[s][0][:, b0[s][3]], b0[s][1][:, b0[s][3]])
        nc.scalar.dma_start(wh_sb, wh_bf)
        nc.sync.dma_start(id_sb, ident)

        prev_z = None  # previous step's z tiles (for deferred transpose+evac)

        for t in range(n_steps):
            if t % TB == 0:
                if t > 0:
                    cur, nxt = nxt, [None] * 4
                if t + TB < n_steps:
                    dma_block(t // TB + 1, nxt)
            sb = t % 2   # S/HH buffer parity

            z = [
                zpools[p].tile([BL, 1024], f32, tag=f"z{p}", name=f"z{p}_{t}")
                for p in range(2)
            ]
            # Each pair's entire step is emitted contiguously: the pairs run
            # skewed by ~half a period, and every engine's FIFO is strict
            # in-order, so grouping per pair keeps FIFO order aligned with
            # readiness order (no cross-pair head-of-line blocking).
            for p in range(2):
                # x-projection matmuls first: independent of the recurrence,
                # they fill the PE while the elementwise chain runs
                for rh in range(2):
                    r = p * 2 + rh
                    rev = streams[r][1]
                    tq = (TB - 1 - t % TB) if rev else (t % TB)
                    tl = cur[r]
                    for k in range(3):
                        nc.tensor.matmul(
                            z[p][:, rh * 512 : rh * 512 + G4],
                            tl[:, (tq * 3 + k) * 128 : (tq * 3 + k + 1) * 128],
                            w_sb[:, (r * 4 + k) * G4 : (r * 4 + k + 1) * G4],
                            start=(k == 0),
                            stop=(t == 0 and k == 2),
                            skip_group_check=True,
                        )
                if prev_z is not None:
                    # transpose h(t-1) into dead PSUM cols + evac to SBUF,
                    # between x(t) and h(t) in the PE FIFO
                    for rh in range(2):
                        nc.tensor.transpose(
                            prev_z[p][0:H, rh * 512 + 400 : rh * 512 + 464].bitcast(bf16),
                            HH[p][1 - sb][:, rh * 100 : rh * 100 + 100],
                            id_sb,
                        )
                    hsrc = (
                        prev_z[p][0:H, :]
                        .bitcast(bf16)
                        .rearrange("q (r c) -> q r c", r=2)[:, :, 800:928]
                    )
                    nc.vector.tensor_copy(
                        HT[p][0:H, :].rearrange("q (r c) -> q r c", r=2), hsrc
                    )
                # recurrent matmuls split by gate group: the {i,j,f} columns
                # finish first so the critical-path sigmoid starts while the
                # {o} columns (only needed much later, at HH) still stream
                # (skipped at t=0: h(-1) == 0, z(0) is the x projection alone)
                for rh in range(2) if t > 0 else []:
                    r = p * 2 + rh
                    nc.tensor.matmul(
                        z[p][:, rh * 512 : rh * 512 + 300],
                        HT[p][0:H, rh * 128 : rh * 128 + 128],
                        wh_sb[0:H, r * G4 : r * G4 + 300],
                        start=False,
                        stop=True,
                        skip_group_check=True,
                    )
                for rh in range(2) if t > 0 else []:
                    r = p * 2 + rh
                    nc.tensor.matmul(
                        z[p][:, rh * 512 + 300 : rh * 512 + G4],
                        HT[p][0:H, rh * 128 : rh * 128 + 128],
                        wh_sb[0:H, r * G4 + 300 : (r + 1) * G4],
                        start=False,
                        stop=True,
                        skip_group_check=True,
                    )

                # sigmoid covers all 4 gates (j cols pre-doubled in W); split
                # {i,j,f} / {o} so the c-path starts early; dst AP permutes
                # run-major z -> gate-major S for flat DVE reads
                za = (
                    z[p][:, :]
                    .rearrange("b (r x) -> b r x", r=2)[:, :, 0:300]
                    .rearrange("b r (g c) -> b r g c", g=3)
                )
                sa = (
                    S[p][sb][:, 0:600]
                    .rearrange("b (g r c) -> b r g c", g=3, r=2)
                )
                nc.scalar.activation(sa, za, Sig)
                Sg = S[p][sb]
                # P' = (sig(2zj) - 0.5) * sig(zi)
                nc.vector.scalar_tensor_tensor(
                    PP[p], Sg[:, 200:400], -0.5, Sg[:, 0:200], add, mult
                )
                # F = sig(zf) * c'
                nc.vector.tensor_tensor(FF[p], Sg[:, 400:600], CC[p], mult)
                zb = z[p][:, :].rearrange("b (r x) -> b r x", r=2)[:, :, 300:400]
                sob = Sg[:, 600:800].rearrange("b (r c) -> b r c", r=2)
                nc.scalar.activation(sob, zb, Sig)
                if t == n_steps - 1:
                    # final c straight to f32 (CC itself is no longer needed)
                    nc.vector.tensor_tensor(CCF[p], FF[p], PP[p], add)
                    nc.sync.dma_start(out[:, p * 200 : (p + 1) * 200], CCF[p])
                    continue
                # c2' = F + P'
                nc.vector.tensor_tensor(CC[p], FF[p], PP[p], add)

                # tc = tanh(2*c') = tanh(c)
                nc.scalar.activation(TC[p], CC[p], Tanh, scale=2.0)
                # h = tc * sig(zo)
                nc.vector.tensor_tensor(
                    HH[p][sb], TC[p], Sg[:, 600:800], mult
                )
            if t == n_steps - 1:
                break
            prev_z = z

    nc.compile()
    return nc


def _prep_xt(x_slice):
    """[BL, T, D] fp32 -> [T//TB, 101, TB*3*BL] bf16 block-major tiles.

    tile[n, p, (tq, j, b)] = x[b, n*TB+tq, j*100+p] for p<100; p=100 is the
    baked-in ones row (bias trick).  Each DMA block is a plain 2D copy with
    TB*3*BL*2 contiguous bytes per partition.
    """
    import ml_dtypes

    a = x_slice.transpose(1, 2, 0).reshape(T // TB, TB, 3, 100, BL)
    a = a.transpose(0, 3, 1, 2, 4)  # [n, p, tq, j, b]
    outp = np.zeros((T // TB, KCH, TB, 3, BL), ml_dtypes.bfloat16)
    outp[:, :100] = a.astype(ml_dtypes.bfloat16)
    outp[:, 100] = 1.0
    return outp.reshape(T // TB, KCH, TB * 3 * BL)


def _prep_weights(Ws, bs):
    """Pack 4 runs' [D+H, 4H] weights into [128, 16*400] chunk blocks.

    Gate columns permuted (i,j,f,o) -> (i,f,o,j); the j block (cols 300:400
    after the permute) is scaled by 2 so tanh(zj) = 2*sigmoid(2 zj) - 1 comes
    out of one sigmoid pass.  Chunk-2's row 100 carries the permuted bias
    (+1.0 forget bias on the f block, x2 on the j block).  Also emits the
    recurrent rows (300:400) as bf16 [128, 4*400].
    """
    import ml_dtypes

    perm = np.arange(400)  # TF gate order (i,j,f,o) kept as-is
    w_all = np.zeros((KCH, 16 * G4), ml_dtypes.bfloat16)
    wh_bf = np.zeros((128, 4 * G4), ml_dtypes.bfloat16)
    for u in range(4):
        Wp = np.asarray(Ws[u], np.float32)[:, perm].copy()
        Wp[:, 100:200] *= 2.0
        bp = np.asarray(bs[u], np.float32)[perm].copy()
        bp[100:200] *= 2.0
        for k in range(3):
            blk = w_all[:, (u * 4 + k) * G4 : (u * 4 + k + 1) * G4]
            blk[0:100] = Wp[k * 100 : (k + 1) * 100].astype(ml_dtypes.bfloat16)
        bias_row = bp.copy()
        bias_row[200:300] += FORGET_BIAS
        w_all[100, (u * 4 + 2) * G4 : (u * 4 + 3) * G4] = bias_row.astype(
            ml_dtypes.bfloat16
        )
        wh_bf[0:H, u * G4 : (u + 1) * G4] = Wp[300:400].astype(ml_dtypes.bfloat16)
    return w_all, wh_bf


def _prep_in_maps(inputs):
    import ml_dtypes

    w_all, wh_bf = _prep_weights(
        [inputs["Wp_fw"], inputs["Wp_bw"], inputs["Wh_fw"], inputs["Wh_bw"]],
        [inputs["bp_fw"], inputs["bp_bw"], inputs["bh_fw"], inputs["bh_bw"]],
    )
    ident = np.eye(128, dtype=ml_dtypes.bfloat16)

    in_maps = []
    for c in range(NCORES):
        sl = slice(c * BL, (c + 1) * BL)
        in_maps.append(
            {
                "xt_p": _prep_xt(np.asarray(inputs["premises"][sl], np.float32)),
                "xt_h": _prep_xt(np.asarray(inputs["hypotheses"][sl], np.float32)),
                "w_all": w_all,
                "wh_bf": wh_bf,
                "ident": ident,
            }
        )
    return in_maps


def _post(res):
    # columns are (c_pf, c_pb, c_hf, c_hb) in run order already; state is c/2
    out = np.concatenate([r["out"] for r in res.results], axis=0)
    return 2.0 * out


def kernel(premises, hypotheses, Wp_fw, bp_fw, Wp_bw, bp_bw, Wh_fw, bh_fw, Wh_bw, bh_bw):
    from concourse.bass_utils import run_bass_kernel_spmd

    if "nc" not in _CACHE:
        _CACHE["nc"] = _build_program()
    nc = _CACHE["nc"]

    in_maps = _prep_in_maps(
        {
            "premises": premises,
            "hypotheses": hypotheses,
            "Wp_fw": Wp_fw, "bp_fw": bp_fw,
            "Wp_bw": Wp_bw, "bp_bw": bp_bw,
            "Wh_fw": Wh_fw, "bh_fw": bh_fw,
            "Wh_bw": Wh_bw, "bh_bw": bh_bw,
        }
    )
    res = run_bass_kernel_spmd(nc, in_maps, core_ids=list(range(NCORES)))
    return _post(res)


# revision 28
# speedup vs baseline: 1.1111x; 1.1037x over previous
"""Trainium2 Bass kernel: bidirectional-LSTM final-cell-state encoder.

Problem: 4 independent BasicLSTMCell chains (premise fw/bw, hypothesis fw/bw),
B=1024, T=128, D=300, H=100.  Output [B, 4H] = concat of final cell states.

Strategy (v9)
-------------
* Data-parallel: batch sharded 8 ways -> 128 rows/core; each core runs the 4
  chains as 2 pairs (premise fw+bw, hypothesis fw+bw) of per-run recurrences.
  The kernel is latency-bound on the per-step recurrence cycle (~3.9us).
* z_t [128b, 400g] per run accumulated in PSUM (pair tile [128, 1024] = 2
  banks, double buffered = 8 banks) from 3 pre-transposed bf16 x chunks + 1
  recurrent h^T chunk.  x chunks stream from DRAM with one-block prefetch;
  block 0 and the weights are split across queues to shorten the prologue.
* TF gate order (i, j, f, o) kept; j columns pre-scaled by 2 so sigmoid
  covers tanh(zj) == 2*sigmoid(2*zj) - 1.
* The recurrent matmul and the sigmoid are split by gate group: h@Wh for
  the {i,j,f} columns (N=300) completes first and unblocks sigmoid_a
  {i,j,f}; the {o} columns + sigmoid_b run in the shadow (o is only needed
  at the very end of the chain, for h).
* sigmoid_a writes S in GATE-MAJOR layout [i|j|f|o][run0|run1][100] via a
  strided dst AP (free permute inside the ACT instruction) so downstream
  DVE ops read flat 2D slices.
* Cell state kept as c' = c/2 in bf16 (fp32 only for the final step's
  output) so the tanh identity constants fold away and the DVE ops hit the
  2x 16-bit perf mode:
    P' = (sig(2zj) - 0.5) * sig(zi)      [DVE scalar_tensor_tensor]
    F  = sig(zf) * c'                    [DVE tensor_tensor, 2x]
    c2' = F + P'                         [DVE tensor_tensor, 2x]
    tc = tanh(2 * c2')                   [ACT, scale=2]
    h  = tc * sig(zo)                    [DVE tensor_tensor, 2x]
  Host multiplies the gathered output by 2.
* h^T: bf16 PE transpose into dead PSUM columns of the z tile, evacuated by
  DVE; transpose+evac of step t-1 are issued between step t's x-matmuls and
  h-matmuls so the PE FIFO never parks a dependent transpose in front of
  independent x-projection work.
"""

import numpy as np

B, T, D, H = 1024, 128, 300, 100
NCORES = 8
BL = B // NCORES          # 128 batch rows per core
G4 = 4 * H                # 400 gate columns
KCH = 128                 # 100 d-rows + 1 ones-row + zero-pad (FWL needs K=128)
TB = 8                    # timesteps per DMA block
FORGET_BIAS = 1.0

_CACHE = {}


def _build_program(n_steps=T):
    from contextlib import ExitStack

    import concourse.mybir as mybir
    import concourse.tile as tile
    from concourse import bacc

    f32 = mybir.dt.float32
    bf16 = mybir.dt.bfloat16
    fp16 = mybir.dt.float16
    Sig = mybir.ActivationFunctionType.Sigmoid
    Tanh = mybir.ActivationFunctionType.Tanh
    mult = mybir.AluOpType.mult
    add = mybir.AluOpType.add

    nc = bacc.Bacc(
        "TRN2",
        target_bir_lowering=False,
        debug=False,
        enable_asserts=False,
        num_devices=NCORES,
    )

    xt_p = nc.dram_tensor("xt_p", [T // TB, KCH, TB * 3 * BL], bf16, kind="ExternalInput").ap()
    xt_h = nc.dram_tensor("xt_h", [T // TB, KCH, TB * 3 * BL], bf16, kind="ExternalInput").ap()
    w_all = nc.dram_tensor("w_all", [KCH, 12 * G4], bf16, kind="ExternalInput").ap()
    wh_bf = nc.dram_tensor("wh_bf", [128, 4 * G4], bf16, kind="ExternalInput").ap()
    ident = nc.dram_tensor("ident", [128, 128], bf16, kind="ExternalInput").ap()
    out = nc.dram_tensor("out", [BL, G4], f32, kind="ExternalOutput").ap()

    with tile.TileContext(nc) as tc, ExitStack() as ctx:
        w_sb = nc.alloc_sbuf_tensor("w_sb", [KCH, 12 * G4], bf16).ap()
        wh_sb = nc.alloc_sbuf_tensor("wh_sb", [128, 4 * G4], bf16).ap()
        id_sb = nc.alloc_sbuf_tensor("id_sb", [128, 128], bf16).ap()

        # per-pair tensors; S and HH double-buffered (cross-engine WAR)
        # S layout is GATE-MAJOR: [i(200) | f(200) | o(200) | j(200)],
        # each gate block = [run0(100) | run1(100)].
        S, PP, FF, CC, TC, HH = [], [], [], [], [], []
        for p in range(2):
            S.append([nc.alloc_sbuf_tensor(f"s{p}_{b}", [BL, 800], bf16).ap() for b in range(2)])
            PP.append(nc.alloc_sbuf_tensor(f"pp{p}", [BL, 200], bf16).ap())
            FF.append(nc.alloc_sbuf_tensor(f"ff{p}", [BL, 200], bf16).ap())
            CC.append(nc.alloc_sbuf_tensor(f"cc{p}", [BL, 200], bf16).ap())
            TC.append(nc.alloc_sbuf_tensor(f"tc{p}", [BL, 200], bf16).ap())
            HH.append([nc.alloc_sbuf_tensor(f"hh{p}_{b}", [BL, 200], bf16).ap() for b in range(2)])
        HT = [nc.alloc_sbuf_tensor(f"ht{p}", [128, 256], bf16).ap() for p in range(2)]
        CCF = [nc.alloc_sbuf_tensor(f"ccf{p}", [BL, 200], f32).ap() for p in range(2)]

        for p in range(2):
            nc.vector.memset(CC[p], 0.0)
        for p in range(2):
            nc.vector.memset(HT[p], 0.0)
        # w rows 101:128 multiply guaranteed-zero x rows; only rows 0:101
        # are DMA'd, but the moving operand reads all 128 - keep them finite
        # (32-aligned partition base; the DMA then overwrites rows 96:101)
        nc.vector.memset(w_sb[96:128, :], 0.0)

        xt_pools = [
            ctx.enter_context(tc.tile_pool(name=f"xt{s}", bufs=2)) for s in range(4)
        ]
        zpools = [
            ctx.enter_context(tc.tile_pool(name=f"zp{p}", bufs=2, space="PSUM"))
            for p in range(2)
        ]

        # stream s: (dram tensor, reversed?) for runs (p_fw, p_bw, h_fw, h_bw)
        streams = [(xt_p, False), (xt_p, True), (xt_h, False), (xt_h, True)]
        cur = [None] * 4
        nxt = [None] * 4

        def dma_block(bi, into):
            for s, (dram, rev) in enumerate(streams):
                tl = xt_pools[s].tile(
                    [KCH, TB * 3 * 128], bf16, tag=f"x{s}", name=f"x{s}_b{bi}"
                )
                nblk = (T // TB - 1 - bi) if rev else bi
                nc.sync.dma_start(tl[:, :], dram[nblk])
                into[s] = tl

        # ── prologue DMAs in priority order across both HWDGE queues ──
        # pair A's step-0 data (w run0/run1 + first-consumed x slices of
        # streams 0/1) first, then pair B's, then the bulk + wh/id.  fw runs
        # read block-0 tq=0.., bw runs read tq=7.., so block 0 is split.
        b0 = []
        for s, (dram, rev) in enumerate(streams):
            tl = xt_pools[s].tile(
                [KCH, TB * 3 * 128], bf16, tag=f"x{s}", name=f"x{s}_b0"
            )
            nblk = (T // TB - 1) if rev else 0
            cut = 2 * 3 * 128 if not rev else 6 * 3 * 128
            first = slice(0, cut) if not rev else slice(cut, TB * 3 * 128)
            rest = slice(cut, TB * 3 * 128) if not rev else slice(0, cut)
            b0.append((tl, dram[nblk], first, rest))
            cur[s] = tl

        def wdma(eng, r):
            eng.dma_start(
                w_sb[0:101, r * 3 * G4 : (r + 1) * 3 * G4],
                w_all[0:101, r * 3 * G4 : (r + 1) * 3 * G4],
            )

        wdma(nc.sync, 0)
        wdma(nc.scalar, 1)
        nc.sync.dma_start(b0[0][0][:, b0[0][2]], b0[0][1][:, b0[0][2]])
        nc.scalar.dma_start(b0[1][0][:, b0[1][2]], b0[1][1][:, b0[1][2]])
        wdma(nc.sync, 2)
        wdma(nc.scalar, 3)
        nc.sync.dma_start(b0[2][0][:, b0[2][2]], b0[2][1][:, b0[2][2]])
        nc.scalar.dma_start(b0[3][0][:, b0[3][2]], b0[3][1][:, b0[3][2]])
        for s in range(4):
            eng = nc.sync if s % 2 == 0 else nc.scalar
            eng.dma_start(b0[s][0][:, b0[s][3]], b0[s][1][:, b0[s][3]])
        nc.scalar.dma_start(wh_sb, wh_bf)
        nc.sync.dma_start(id_sb, ident)

        prev_z = None  # previous step's z tiles (for deferred transpose+evac)

        for t in range(n_steps):
            if t % TB == 0:
                if t > 0:
                    cur, nxt = nxt, [None] * 4
                if t + TB < n_steps:
                    dma_block(t // TB + 1, nxt)
            sb = t % 2   # S/HH buffer parity

            z = [
                zpools[p].tile([BL, 1024], f32, tag=f"z{p}", name=f"z{p}_{t}")
                for p in range(2)
            ]
            # Each pair's entire step is emitted contiguously: the pairs run
            # skewed by ~half a period, and every engine's FIFO is strict
            # in-order, so grouping per pair keeps FIFO order aligned with
            # readiness order (no cross-pair head-of-line blocking).
            for p in range(2):
                # x-projection matmuls first: independent of the recurrence,
                # they fill the PE while the elementwise chain runs
                for rh in range(2):
                    r = p * 2 + rh
                    rev = streams[r][1]
                    tq = (TB - 1 - t % TB) if rev else (t % TB)
                    tl = cur[r]
                    for k in range(3):
                        nc.tensor.matmul(
                            z[p][:, rh * 512 : rh * 512 + G4],
                            tl[:, (tq * 3 + k) * 128 : (tq * 3 + k + 1) * 128],
                            w_sb[:, (r * 3 + k) * G4 : (r * 3 + k + 1) * G4],
                            start=(k == 0),
                            stop=(t == 0 and k == 2),
                            skip_group_check=True,
                        )
                if prev_z is not None:
                    # transpose h(t-1) into dead PSUM cols + evac to SBUF,
                    # between x(t) and h(t) in the PE FIFO
                    for rh in range(2):
                        nc.tensor.transpose(
                            prev_z[p][0:H, rh * 512 + 400 : rh * 512 + 464].bitcast(bf16),
                            HH[p][1 - sb][:, rh * 100 : rh * 100 + 100],
                            id_sb,
                        )
                    hsrc = (
                        prev_z[p][0:H, :]
                        .bitcast(bf16)
                        .rearrange("q (r c) -> q r c", r=2)[:, :, 800:928]
                    )
                    nc.vector.tensor_copy(
                        HT[p][0:H, :].rearrange("q (r c) -> q r c", r=2), hsrc
                    )
                # recurrent matmuls split by gate group: the {i,j,f} columns
                # finish first so the critical-path sigmoid starts while the
                # {o} columns (only needed much later, at HH) still stream
                # (skipped at t=0: h(-1) == 0, z(0) is the x projection alone)
                for rh in range(2) if t > 0 else []:
                    r = p * 2 + rh
                    nc.tensor.matmul(
                        z[p][:, rh * 512 : rh * 512 + 300],
                        HT[p][0:H, rh * 128 : rh * 128 + 128],
                        wh_sb[0:H, r * G4 : r * G4 + 300],
                        start=False,
                        stop=True,
                        skip_group_check=True,
                    )
                for rh in range(2) if t > 0 else []:
                    r = p * 2 + rh
                    nc.tensor.matmul(
                        z[p][:, rh * 512 + 300 : rh * 512 + G4],
                        HT[p][0:H, rh * 128 : rh * 128 + 128],
                        wh_sb[0:H, r * G4 + 300 : (r + 1) * G4],
                        start=False,
                        stop=True,
                        skip_group_check=True,
                    )

                # sigmoid covers all 4 gates (j cols pre-doubled in W); split
                # {i,j,f} / {o} so the c-path starts early; dst AP permutes
                # run-major z -> gate-major S for flat DVE reads
                za = (
                    z[p][:, :]
                    .rearrange("b (r x) -> b r x", r=2)[:, :, 0:300]
                    .rearrange("b r (g c) -> b r g c", g=3)
                )
                sa = (
                    S[p][sb][:, 0:600]
                    .rearrange("b (g r c) -> b r g c", g=3, r=2)
                )
                nc.scalar.activation(sa, za, Sig)
                Sg = S[p][sb]
                # P' = (sig(2zj) - 0.5) * sig(zi)
                nc.vector.scalar_tensor_tensor(
                    PP[p], Sg[:, 200:400], -0.5, Sg[:, 0:200], add, mult
                )
                # F = sig(zf) * c'
                nc.vector.tensor_tensor(FF[p], Sg[:, 400:600], CC[p], mult)
                zb = z[p][:, :].rearrange("b (r x) -> b r x", r=2)[:, :, 300:400]
                sob = Sg[:, 600:800].rearrange("b (r c) -> b r c", r=2)
                nc.scalar.activation(sob, zb, Sig)
                if t == n_steps - 1:
                    # final c straight to f32 (CC itself is no longer needed)
                    nc.vector.tensor_tensor(CCF[p], FF[p], PP[p], add)
                    nc.sync.dma_start(out[:, p * 200 : (p + 1) * 200], CCF[p])
                    continue
                # c2' = F + P'
                nc.vector.tensor_tensor(CC[p], FF[p], PP[p], add)

                # tc = tanh(2*c') = tanh(c)
                nc.scalar.activation(TC[p], CC[p], Tanh, scale=2.0)
                # h = tc * sig(zo)
                nc.vector.tensor_tensor(
                    HH[p][sb], TC[p], Sg[:, 600:800], mult
                )
            if t == n_steps - 1:
                break
            prev_z = z

    nc.compile()
    return nc


def _prep_xt(x_slice):
    """[BL, T, D] fp32 -> [T//TB, 101, TB*3*BL] bf16 block-major tiles.

    tile[n, p, (tq, j, b)] = x[b, n*TB+tq, j*100+p] for p<100; p=100 is the
    baked-in ones row (bias trick).  Each DMA block is a plain 2D copy with
    TB*3*BL*2 contiguous bytes per partition.
    """
    import ml_dtypes

    a = x_slice.transpose(1, 2, 0).reshape(T // TB, TB, 3, 100, BL)
    a = a.transpose(0, 3, 1, 2, 4)  # [n, p, tq, j, b]
    outp = np.zeros((T // TB, KCH, TB, 3, BL), ml_dtypes.bfloat16)
    outp[:, :100] = a.astype(ml_dtypes.bfloat16)
    outp[:, 100] = 1.0
    return outp.reshape(T // TB, KCH, TB * 3 * BL)


def _prep_weights(Ws, bs):
    """Pack 4 runs' [D+H, 4H] weights into [128, 16*400] chunk blocks.

    Gate columns permuted (i,j,f,o) -> (i,f,o,j); the j block (cols 300:400
    after the permute) is scaled by 2 so tanh(zj) = 2*sigmoid(2 zj) - 1 comes
    out of one sigmoid pass.  Chunk-2's row 100 carries the permuted bias
    (+1.0 forget bias on the f block, x2 on the j block).  Also emits the
    recurrent rows (300:400) as bf16 [128, 4*400].
    """
    import ml_dtypes

    perm = np.arange(400)  # TF gate order (i,j,f,o) kept as-is
    w_all = np.zeros((KCH, 12 * G4), ml_dtypes.bfloat16)
    wh_bf = np.zeros((128, 4 * G4), ml_dtypes.bfloat16)
    for u in range(4):
        Wp = np.asarray(Ws[u], np.float32)[:, perm].copy()
        Wp[:, 100:200] *= 2.0
        bp = np.asarray(bs[u], np.float32)[perm].copy()
        bp[100:200] *= 2.0
        for k in range(3):
            blk = w_all[:, (u * 3 + k) * G4 : (u * 3 + k + 1) * G4]
            blk[0:100] = Wp[k * 100 : (k + 1) * 100].astype(ml_dtypes.bfloat16)
        bias_row = bp.copy()
        bias_row[200:300] += FORGET_BIAS
        w_all[100, (u * 3 + 2) * G4 : (u * 3 + 3) * G4] = bias_row.astype(
            ml_dtypes.bfloat16
        )
        wh_bf[0:H, u * G4 : (u + 1) * G4] = Wp[300:400].astype(ml_dtypes.bfloat16)
    return w_all, wh_bf


def _prep_in_maps(inputs):
    import ml_dtypes

    w_all, wh_bf = _prep_weights(
        [inputs["Wp_fw"], inputs["Wp_bw"], inputs["Wh_fw"], inputs["Wh_bw"]],
        [inputs["bp_fw"], inputs["bp_bw"], inputs["bh_fw"], inputs["bh_bw"]],
    )
    ident = np.eye(128, dtype=ml_dtypes.bfloat16)

    in_maps = []
    for c in range(NCORES):
        sl = slice(c * BL, (c + 1) * BL)
        in_maps.append(
            {
                "xt_p": _prep_xt(np.asarray(inputs["premises"][sl], np.float32)),
                "xt_h": _prep_xt(np.asarray(inputs["hypotheses"][sl], np.float32)),
                "w_all": w_all,
                "wh_bf": wh_bf,
                "ident": ident,
            }
        )
    return in_maps


def _post(res):
    # columns are (c_pf, c_pb, c_hf, c_hb) in run order already; state is c/2
    out = np.concatenate([r["out"] for r in res.results], axis=0)
    return 2.0 * out


def kernel(premises, hypotheses, Wp_fw, bp_fw, Wp_bw, bp_bw, Wh_fw, bh_fw, Wh_bw, bh_bw):
    from concourse.bass_utils import run_bass_kernel_spmd

    if "nc" not in _CACHE:
        _CACHE["nc"] = _build_program()
    nc = _CACHE["nc"]

    in_maps = _prep_in_maps(
        {
            "premises": premises,
            "hypotheses": hypotheses,
            "Wp_fw": Wp_fw, "bp_fw": bp_fw,
            "Wp_bw": Wp_bw, "bp_bw": bp_bw,
            "Wh_fw": Wh_fw, "bh_fw": bh_fw,
            "Wh_bw": Wh_bw, "bh_bw": bh_bw,
        }
    )
    res = run_bass_kernel_spmd(nc, in_maps, core_ids=list(range(NCORES)))
    return _post(res)
